# revision 24
# baseline (speedup 1.0000x reference)
"""Trainium2 Bass kernel for nn_Attention_17008070493108.

Dense transformer attention block: QKV proj -> per-head LayerNorm -> RoPE
-> SDPA -> out proj, for x[2, 2048, 1024], H=16 heads, head_dim=64.

Sharding: tensor-parallel over heads. Each of the 8 NeuronCores owns 2
heads end-to-end (QKV column slices, norm, RoPE, attention). The
per-head context vectors are exchanged with a single AllToAll so each
core finishes the output projection (contraction over the full 1024
model dims) for its own 512-row slice of the output; the host
concatenates row slices.

Layouts (per core):
  xT          [1024, 4096] model-dim on partitions (host-transposed x)
  QT/KT       [128, 2048]x2 (batch-split) heads stacked on partitions
  scoresT     [128 keys, q] key tiles on partitions; softmax denominator
                          via a ones-column appended to V (ctx_aug row 64)
  ctx         [65, 512] psum x4 -> normalize -> ctxn [128, 4096] bf16
              -> AllToAll -> out rows [512, 1024]

Fast paths vs the original emission:
  * LayerNorm stats -> Rsqrt activation (no DVE reciprocal), kept in
    SBUF for batch 0; the per-column rstd/mu*rstd broadcasts for the
    LN apply are rank-2 PE matmuls (lhsT = LN-weight-masked ones) so
    the LN weight is folded in for free, and (x*wr + b - w*mu*r) is a
    single fused scalar_tensor_tensor op.
  * The RoPE half-swap is a 128x128 permutation matmul instead of four
    SBUF-to-SBUF DMAs.
  * Softmax normalize uses reciprocal_approx_fast directly on the psum
    denominator row (no DRAM roundtrips) and multiplies psum ctx rows
    straight into the bf16 AllToAll staging tile.
  * Wo is preloaded at kernel start; host pre-permutes wqkv/onesblk/wo
    so every constant DMA is one descriptor per partition.
Batch-1 LN applies (which overlap SDPA, where all 8 PSUM banks are
busy) keep the gpsimd partition_broadcast path.
"""

import numpy as np

from concourse import bacc, tile, mybir
from concourse.bass_utils import run_bass_kernel_spmd

# ---------------------------------------------------------------- constants
DIM = 1024
H = 16
HD = 64
B = 2
N = 2048
R = B * N          # 4096 flattened rows
NCORE = 8
EPS = 1e-6

F32 = mybir.dt.float32
F32R = mybir.dt.float32r
BF16 = mybir.dt.bfloat16
ADD = mybir.AluOpType.add
SUB = mybir.AluOpType.subtract
MUL = mybir.AluOpType.mult

RC = R // 512        # 8 row chunks of 512
KT_DIM = DIM // 128  # 8 contraction tiles for the projections
NQC = N // 512       # 4 q chunks per batch
NKT = N // 128       # 16 key tiles per batch
VSTRIDE = 130        # per-keytile V_aug block: [vA(64) | 1 | vB(64) | 1]

DEBUG_OUTPUTS = False


def _round_fp32r(x: np.ndarray) -> np.ndarray:
    """Round fp32 to fp32r (11-bit mantissa, RNE)."""
    u = np.ascontiguousarray(x, dtype=np.float32).view(np.uint32)
    lsb = (u >> np.uint32(12)) & np.uint32(1)
    r = (u + np.uint32(0x7FF) + lsb) & np.uint32(0xFFFFF000)
    return r.view(np.float32)


# ---------------------------------------------------------------- graph
def build():
    nc = bacc.Bacc("TRN2", target_bir_lowering=False, debug=False,
                   num_devices=NCORE)

    # ---- DRAM parameters (host pre-permuted for contiguous DMA)
    xT_d = nc.dram_tensor("xT", [DIM, R], F32R, kind="ExternalInput")
    wqkv_d = nc.dram_tensor("wqkv", [128, KT_DIM, 384], F32R,
                            kind="ExternalInput")
    bqkv_d = nc.dram_tensor("bqkv", [3, 128, 1], F32, kind="ExternalInput")
    # stats lhsT: [:, :, 0, c] x-sums col {2jj+h}, [:, :, 1, c] sq-sums
    # col {32+2jj+h} (offset 32 keeps DVE reads partition-aligned); both
    # accumulate into one [40, 512] psum bank.
    onesblk_d = nc.dram_tensor("onesblk", [128, RC, 2, 40], F32R,
                               kind="ExternalInput")
    lnb_d = nc.dram_tensor("lnb", [2, 128, 1], F32, kind="ExternalInput")
    lnw_d = nc.dram_tensor("lnw", [2, 128, 1], F32, kind="ExternalInput")
    wbc_d = nc.dram_tensor("wbc", [8, 2, 4, 128], F32R,
                           kind="ExternalInput")
    perm_d = nc.dram_tensor("perm", [128, 128], F32R, kind="ExternalInput")
    cos_d = nc.dram_tensor("cosr", [128, R], F32, kind="ExternalInput")
    sinm_d = nc.dram_tensor("sinm", [128, R], F32, kind="ExternalInput")
    ident_d = nc.dram_tensor("ident", [128, 128], F32, kind="ExternalInput")
    ones_d = nc.dram_tensor("ones64", [128, 4 * NKT], F32R,
                            kind="ExternalInput")
    wo_d = nc.dram_tensor("wo", [128, KT_DIM, DIM], BF16,
                          kind="ExternalInput")
    borep_d = nc.dram_tensor("borep", [128, DIM], F32, kind="ExternalInput")
    out_d = nc.dram_tensor("out", [R // NCORE, DIM], F32, kind="ExternalOutput")
    if DEBUG_OUTPUTS:
        dbg_qrot = nc.dram_tensor("dbg_qrot", [128, R], F32,
                                  kind="ExternalOutput")
        dbg_krot = nc.dram_tensor("dbg_krot", [128, R], F32,
                                  kind="ExternalOutput")
        dbg_ctxn = nc.dram_tensor("dbg_ctxn", [128, R], BF16,
                                  kind="ExternalOutput")

    with tile.TileContext(nc) as tc:
        with (
            tc.tile_pool(name="const", bufs=1) as cpool,
            tc.tile_pool(name="persist", bufs=1) as ppool,
            tc.tile_pool(name="chp", bufs=2) as chpool,
            tc.tile_pool(name="statp", bufs=6) as statpool,
            tc.tile_pool(name="gpp", bufs=1) as gppool,
            tc.tile_pool(name="stagp", bufs=4) as stagpool,
            tc.tile_pool(name="dram", bufs=1, space="DRAM") as dpool,
        ):
            # ---- constants in SBUF (contiguous per-partition DMAs)
            # q/k weight columns first so row-0 projection starts ASAP
            wqkv_sb = cpool.tile([128, KT_DIM, 384], F32R)
            nc.sync.dma_start(out=wqkv_sb[:, :, 0:128],
                              in_=wqkv_d.ap()[:, :, 0:128])
            bq_sb = cpool.tile([128, 1], F32)
            bk_sb = cpool.tile([128, 1], F32)
            bv_sb = cpool.tile([128, 1], F32)
            nc.sync.dma_start(out=bq_sb[:], in_=bqkv_d.ap()[0])
            nc.sync.dma_start(out=bk_sb[:], in_=bqkv_d.ap()[1])
            nc.sync.dma_start(out=bv_sb[:], in_=bqkv_d.ap()[2])
            onesblk_sb = cpool.tile([128, RC, 2, 40], F32R)
            nc.sync.dma_start(out=onesblk_sb[:], in_=onesblk_d.ap()[:, :, :, :])
            nc.sync.dma_start(out=wqkv_sb[:, :, 128:384],
                              in_=wqkv_d.ap()[:, :, 128:384])
            lnbq_sb = cpool.tile([128, 1], F32)
            lnbk_sb = cpool.tile([128, 1], F32)
            nc.sync.dma_start(out=lnbq_sb[:], in_=lnb_d.ap()[0])
            nc.sync.dma_start(out=lnbk_sb[:], in_=lnb_d.ap()[1])
            lnwq_sb = cpool.tile([128, 1], F32)
            lnwk_sb = cpool.tile([128, 1], F32)
            nc.sync.dma_start(out=lnwq_sb[:], in_=lnw_d.ap()[0])
            nc.sync.dma_start(out=lnwk_sb[:], in_=lnw_d.ap()[1])
            wbc_sb = cpool.tile([8, 2, 4, 128], F32R)
            nc.sync.dma_start(out=wbc_sb[:], in_=wbc_d.ap()[:, :, :, :])
            perm_sb = cpool.tile([128, 128], F32R)
            nc.sync.dma_start(out=perm_sb[:], in_=perm_d.ap()[:, :])
            ident_sb = cpool.tile([128, 128], F32)
            nc.sync.dma_start(out=ident_sb[:], in_=ident_d.ap()[:, :])
            borep_sb = cpool.tile([128, DIM], F32)
            wo_sb = cpool.tile([128, KT_DIM, DIM], BF16)

            # ---- persistent tensors (batch-split Q/K; in-place LN+RoPE)
            qkt = {}
            for g in range(B):
                qkt[("q", g)] = ppool.tile([128, N], F32R, tag=f"q{g}",
                                           name=f"qraw{g}")
                qkt[("k", g)] = ppool.tile([128, N], F32R, tag=f"k{g}",
                                           name=f"kraw{g}")
            vaug = ppool.tile([128, 2 * NKT * VSTRIDE], F32R, tag="vaug")
            ctxn_a = ppool.tile([64, R], BF16, tag="ctxn_a")
            ctxn_b = ppool.tile([64, R], BF16, tag="ctxn_b")
            # batch-0 stats kept in SBUF for the PE-broadcast LN path
            stat_sb = {}
            for tname in ("q", "k"):
                stat_sb[(tname, "r")] = ppool.tile(
                    [8, 512], F32R, tag=f"rstd_{tname}",
                    name=f"rstd_{tname}")
                stat_sb[(tname, "m")] = ppool.tile(
                    [8, 512], F32R, tag=f"mur_{tname}", name=f"mur_{tname}")

            nc.gpsimd.dma_start(
                out=vaug[:].rearrange("p (k c) -> p k c", c=65)[:, :, 64:65],
                in_=ones_d.ap()[:, :])

            stat_dr = {}

            # ---------------- emission helpers ----------------
            def emit_proj_load(r, xtpool):
                """DMA the 8 xT contraction tiles for row-chunk r."""
                xts = []
                for kt in range(KT_DIM):
                    xt = xtpool.tile([128, 512], F32R, tag="xt",
                                     name=f"xt_{r}_{kt}")
                    nc.sync.dma_start(
                        out=xt[:],
                        in_=xT_d.ap()[kt * 128:(kt + 1) * 128,
                                      r * 512:(r + 1) * 512])
                    xts.append(xt)
                return xts

            def emit_proj_m(r, m, xts, vchpool, ps1, ps1v, statps):
                """Project row-chunk r for one of q/k/v (+ stats MMs)."""
                g, jj = r // 4, r % 4
                name, bias = (("q", bq_sb), ("k", bk_sb),
                              ("v", bv_sb))[m]
                ps = ps1.tile([128, 512], F32, tag="proj",
                              name=f"proj_{m}_{r}")
                for kt in range(KT_DIM):
                    nc.tensor.matmul(
                        ps[:], wqkv_sb[:, kt, m * 128:(m + 1) * 128],
                        xts[kt][:],
                        start=(kt == 0), stop=(kt == KT_DIM - 1))
                if m < 2:
                    dest = qkt[(name, g)]
                    nc.vector.tensor_scalar(
                        dest[:, jj * 512:(jj + 1) * 512], ps[:],
                        bias[:], None, ADD)
                    sps = statps[(name, g)]
                    nc.tensor.matmul(
                        sps[:], onesblk_sb[:, r, 0, :],
                        dest[:, jj * 512:(jj + 1) * 512],
                        start=(jj == 0), stop=False)
                    sqc = chpool.tile([128, 512], F32R, tag="sqc",
                                      name=f"sqc_{name}_{r}")
                    nc.scalar.square(
                        sqc[:],
                        dest[:, jj * 512:(jj + 1) * 512].bitcast(F32))
                    nc.tensor.matmul(
                        sps[:], onesblk_sb[:, r, 1, :], sqc[:],
                        start=False, stop=(jj == 3))
                else:
                    vch = vchpool.tile([128, 512], F32, tag="vch",
                                       name=f"vch_{r}")
                    nc.scalar.add(vch[:], ps[:], bias[:])
                    for sseg in range(4):
                        kt_glob = r * 4 + sseg
                        tps = ps1v.tile([128, 128], F32, tag="vtr",
                                        name=f"vtr_{kt_glob}")
                        nc.tensor.transpose(
                            tps[:], vch[:, sseg * 128:(sseg + 1) * 128],
                            ident_sb[:])
                        vb = kt_glob * VSTRIDE
                        nc.scalar.copy(
                            vaug[:, vb:vb + 64], tps[:, 0:64])
                        nc.scalar.copy(
                            vaug[:, vb + 65:vb + 129], tps[:, 64:128])

            def emit_proj_row(r, xtpool, vchpool, ps1, ps1v, statps):
                xts = emit_proj_load(r, xtpool)
                for m in range(3):
                    emit_proj_m(r, m, xts, vchpool, ps1, ps1v, statps)

            def emit_statmath(name, g, statps):
                """stat bank [40, 512]: rows 0-7 x-sums, 32-39 sq-sums.
                g=0: rstd/mu*rstd -> persistent SBUF (PE-broadcast path).
                g=1: staged to DRAM for the gpsimd-broadcast path."""
                sps = statps[(name, g)]
                mu = statpool.tile([8, 512], F32, tag="stat_sb",
                                   name=f"mu_{name}{g}")
                msqe = statpool.tile([8, 512], F32, tag="stat_sb",
                                     name=f"msqe_{name}{g}")
                nc.vector.tensor_scalar(mu[:], sps[0:8, :], 1.0 / HD,
                                        None, MUL)
                nc.vector.tensor_scalar(msqe[:], sps[32:40, :], 1.0 / HD,
                                        EPS, MUL, ADD)
                var = statpool.tile([8, 512], F32, tag="stat_sb",
                                    name=f"var_{name}{g}")
                nc.vector.tensor_tensor(var[:], mu[:], mu[:], MUL)
                nc.vector.tensor_tensor(var[:], msqe[:], var[:], SUB)
                sd = statpool.tile([8, 512], F32, tag="stat_sb",
                                   name=f"sd_{name}{g}")
                nc.scalar.activation(sd[:], var[:],
                                     mybir.ActivationFunctionType.Sqrt)
                if g == 0:
                    rtmp = statpool.tile([8, 512], F32, tag="stat_sb",
                                         name=f"rtmp_{name}{g}")
                    nc.vector.reciprocal_approx_fast(rtmp[:], sd[:])
                    # DVE writes to F32R tiles round properly for the
                    # downstream fp32r rep matmuls
                    nc.vector.tensor_copy(stat_sb[(name, "r")][:], rtmp[:])
                    nc.vector.tensor_tensor(stat_sb[(name, "m")][:], mu[:],
                                            rtmp[:], MUL)
                else:
                    rstd = statpool.tile([8, 512], F32, tag="stat_sb",
                                         name=f"rstd_{name}{g}")
                    nc.vector.reciprocal_approx_fast(rstd[:], sd[:])
                    murstd = statpool.tile([8, 512], F32, tag="stat_sb",
                                           name=f"murstd_{name}{g}")
                    nc.vector.tensor_tensor(murstd[:], mu[:],
                                            rstd[:], MUL)
                    rdr = dpool.tile([8, 512], F32, name=f"rstd_dr_{name}{g}")
                    mdr = dpool.tile([8, 512], F32, name=f"mur_dr_{name}{g}")
                    nc.sync.dma_start(out=rdr[:], in_=rstd[:])
                    nc.sync.dma_start(out=mdr[:], in_=murstd[:])
                    stat_dr[(name, g)] = (rdr, mdr)

            def emit_cs_load(jj):
                """cos/sin for batch-0 chunk jj (shared by q and k)."""
                jsl = slice(jj * 512, (jj + 1) * 512)
                cosc = chpool.tile([128, 512], F32, tag="cosc",
                                   name=f"cosc_0{jj}")
                sinc = chpool.tile([128, 512], F32, tag="sinc",
                                   name=f"sinc_0{jj}")
                nc.sync.dma_start(out=cosc[:], in_=cos_d.ap()[:, jsl])
                nc.sync.dma_start(out=sinc[:], in_=sinm_d.ap()[:, jsl])
                return cosc, sinc

            def emit_apply_pe_head(name, jj, wi, b_sb, reppool):
                """LN apply for batch-0 chunk jj: rep matmuls + fused DVE."""
                traw = qkt[(name, 0)]
                rstd = stat_sb[(name, "r")]
                murstd = stat_sb[(name, "m")]
                jsl = slice(jj * 512, (jj + 1) * 512)
                rep = reppool.tile([128, 1024], F32, tag="rep",
                                   name=f"rep_{name}_0{jj}")
                nc.tensor.matmul(rep[:, 0:512], wbc_sb[:, wi, jj, :],
                                 rstd[:], start=True, stop=True)
                nc.tensor.matmul(rep[:, 512:1024], wbc_sb[:, wi, jj, :],
                                 murstd[:], start=True, stop=True)
                tn = chpool.tile([128, 512], F32R, tag="tn",
                                 name=f"tn_{name}_0{jj}")
                nc.vector.tensor_tensor(tn[:], traw[:, jsl].bitcast(F32),
                                        rep[:, 0:512], MUL)
                nc.vector.scalar_tensor_tensor(
                    tn[:], tn[:].bitcast(F32), b_sb[:], rep[:, 512:1024],
                    ADD, SUB)
                return tn

            def emit_apply_pe_tail(name, jj, tn, cosc, sinc, swppool):
                """RoPE for batch-0 chunk jj: perm matmul + rotate-combine."""
                traw = qkt[(name, 0)]
                jsl = slice(jj * 512, (jj + 1) * 512)
                swp = swppool.tile([128, 512], F32, tag="swp",
                                   name=f"swp_{name}_0{jj}")
                nc.tensor.matmul(swp[:], perm_sb[:], tn[:],
                                 start=True, stop=True)
                t1 = chpool.tile([128, 512], F32, tag="t1",
                                 name=f"t1_{name}_0{jj}")
                nc.gpsimd.tensor_tensor(t1[:], tn[:].bitcast(F32), cosc[:],
                                        MUL)
                s2 = chpool.tile([128, 512], F32, tag="t1",
                                 name=f"s2_{name}_0{jj}")
                nc.vector.tensor_tensor(s2[:], swp[:], sinc[:], MUL)
                nc.vector.tensor_tensor(traw[:, jsl], t1[:], s2[:], ADD)

            def emit_apply_gp(name, jj, w_sb, b_sb):
                """Batch-1 LN apply + RoPE (gpsimd broadcasts, no PSUM;
                runs overlapped with SDPA where all 8 banks are taken)."""
                g = 1
                traw = qkt[(name, g)]
                rdr, mdr = stat_dr[(name, g)]
                jsl = slice(jj * 512, (jj + 1) * 512)
                gsl = slice(g * N + jj * 512, g * N + (jj + 1) * 512)
                cosc = chpool.tile([128, 512], F32, tag="cosc",
                                   name=f"cosc_{name}_{g}{jj}")
                sinc = chpool.tile([128, 512], F32, tag="sinc",
                                   name=f"sinc_{name}_{g}{jj}")
                nc.sync.dma_start(out=cosc[:], in_=cos_d.ap()[:, gsl])
                nc.sync.dma_start(out=sinc[:], in_=sinm_d.ap()[:, gsl])
                rep_r = gppool.tile([128, 512], F32, tag="rep_r",
                                    name=f"rep_r_{name}_{g}{jj}")
                rep_m = gppool.tile([128, 512], F32, tag="rep_m",
                                    name=f"rep_m_{name}_{g}{jj}")
                for h in range(2):
                    stg_r = stagpool.tile([1, 512], F32, tag="stg",
                                          name=f"sr_{name}_{g}{jj}_{h}")
                    stg_m = stagpool.tile([1, 512], F32, tag="stg",
                                          name=f"sm_{name}_{g}{jj}_{h}")
                    nc.sync.dma_start(out=stg_r[:], in_=rdr[2 * jj + h])
                    nc.sync.dma_start(out=stg_m[:], in_=mdr[2 * jj + h])
                    if h == 0:
                        nc.gpsimd.partition_broadcast(
                            rep_r[0:64, :], stg_r[:], channels=64)
                        nc.gpsimd.partition_broadcast(
                            rep_m[0:64, :], stg_m[:], channels=64)
                    else:
                        tmp_r = stagpool.tile([64, 512], F32, tag="tmpb",
                                              name=f"tr_{name}_{g}{jj}")
                        tmp_m = stagpool.tile([64, 512], F32, tag="tmpb",
                                              name=f"tm_{name}_{g}{jj}")
                        nc.gpsimd.partition_broadcast(
                            tmp_r[:], stg_r[:], channels=64)
                        nc.gpsimd.partition_broadcast(
                            tmp_m[:], stg_m[:], channels=64)
                        nc.sync.dma_start(out=rep_r[64:128, :],
                                          in_=tmp_r[:])
                        nc.sync.dma_start(out=rep_m[64:128, :],
                                          in_=tmp_m[:])
                tn = chpool.tile([128, 512], F32, tag="tn",
                                 name=f"tn_{name}_{g}{jj}")
                nc.vector.tensor_tensor(tn[:], traw[:, jsl].bitcast(F32),
                                        rep_r[:], MUL)
                nc.vector.tensor_tensor(tn[:], tn[:], rep_m[:], SUB)
                nc.vector.tensor_scalar(tn[:], tn[:], w_sb[:], b_sb[:],
                                        MUL, ADD)
                swp = gppool.tile([128, 512], F32, tag="swp",
                                  name=f"swp_{name}_{g}{jj}")
                for (dst, src) in ((0, 32), (32, 0), (64, 96), (96, 64)):
                    nc.sync.dma_start(out=swp[dst:dst + 32, :],
                                      in_=tn[src:src + 32, :])
                t1 = chpool.tile([128, 512], F32, tag="t1",
                                 name=f"t1_{name}_{g}{jj}")
                nc.vector.tensor_tensor(t1[:], tn[:], cosc[:], MUL)
                nc.vector.tensor_tensor(swp[:], swp[:], sinc[:], MUL)
                nc.vector.tensor_tensor(traw[:, jsl], t1[:], swp[:], ADD)

            def make_sdpa(exppool, sp3, sp4, ps_sc, ps_ctx, a2a_in):
                """Globally software-pipelined SDPA over all qc-pairs:
                QK/exp run one kt-step ahead of PV across pair boundaries
                so the Exp unit never waits for the in-order PE pipe; each
                pair's normalize + AllToAll staging happens as soon as its
                last PV retires."""
                ctxmap = {}

                def emit_qkexp(pi, g, qc0, kt):
                    qrot = qkt[("q", g)]
                    krot = qkt[("k", g)]
                    ksl = slice(kt * 128, (kt + 1) * 128)
                    scs = {}
                    for qc in (qc0, qc0 + 1):
                        scs[qc] = ps_sc.tile([128, 1024], F32, tag="sc",
                                             name=f"sc_{g}{qc}{kt}")
                    for h, psl in ((0, slice(0, 64)), (1, slice(64, 128))):
                        for qc in (qc0, qc0 + 1):
                            qsl = slice(qc * 512, (qc + 1) * 512)
                            nc.tensor.matmul(
                                scs[qc][:, h * 512:(h + 1) * 512],
                                krot[psl, ksl], qrot[psl, qsl],
                                start=True, stop=True,
                                tile_position=(h * 64, 0))
                    exps = {}
                    for qc in (qc0, qc0 + 1):
                        expt = exppool.tile([128, 1024], F32R, tag="expt",
                                            name=f"ex_{g}{qc}{kt}")
                        nc.scalar.activation(
                            expt[:], scs[qc][:],
                            mybir.ActivationFunctionType.Exp,
                            scale=float(HD) ** -0.5)
                        exps[qc] = expt
                    return exps

                def emit_pv(pi, g, qc0, kt, exps):
                    if kt == 0:
                        d = {}
                        for qc in (qc0, qc0 + 1):
                            for h in range(2):
                                d[(qc, h)] = ps_ctx.tile(
                                    [65, 512], F32, tag="ctx",
                                    name=f"ctx_{g}{qc}{h}")
                        ctxmap[pi] = d
                    ctxps = ctxmap[pi]
                    vbase = (g * NKT + kt) * VSTRIDE
                    for h in range(2):
                        vsl = slice(vbase + h * 65, vbase + (h + 1) * 65)
                        for qc in (qc0, qc0 + 1):
                            nc.tensor.matmul(
                                ctxps[(qc, h)][:], vaug[:, vsl],
                                exps[qc][:, h * 512:(h + 1) * 512],
                                start=(kt == 0), stop=(kt == NKT - 1))

                def emit_norm(pi, g, qc0):
                    ctxps = ctxmap.pop(pi)
                    for qc in (qc0, qc0 + 1):
                        qsl = slice(g * N + qc * 512,
                                    g * N + (qc + 1) * 512)
                        for h, dst in ((0, ctxn_a), (1, ctxn_b)):
                            cu = sp4.tile([65, 512], F32, tag="cu",
                                          name=f"cu{g}{qc}{h}")
                            nc.vector.tensor_copy(cu[:], ctxps[(qc, h)][:])
                            dn = sp3.tile([1, 512], F32, tag="dn",
                                          name=f"dn{g}{qc}{h}")
                            nc.sync.dma_start(out=dn[:], in_=cu[64:65, :])
                            rc = sp3.tile([1, 512], F32, tag="rc",
                                          name=f"rc{g}{qc}{h}")
                            nc.vector.reciprocal_approx_fast(rc[:], dn[:])
                            rep = sp3.tile([64, 512], F32, tag="rep",
                                           name=f"rp{g}{qc}{h}")
                            nc.gpsimd.partition_broadcast(rep[:], rc[:],
                                                          channels=64)
                            nc.vector.tensor_tensor(
                                dst[:, qsl], cu[0:64, :], rep[:], MUL)
                        # stage this dest chunk for the AllToAll now
                        j = g * 4 + qc
                        nc.gpsimd.dma_start(
                            out=a2a_in[j][0:64, :],
                            in_=ctxn_a[:, g * N + qc * 512:
                                       g * N + (qc + 1) * 512])
                        nc.gpsimd.dma_start(
                            out=a2a_in[j][64:128, :],
                            in_=ctxn_b[:, g * N + qc * 512:
                                       g * N + (qc + 1) * 512])

                def run(pairs, between):
                    prev = None
                    for pi, (g, qc0) in enumerate(pairs):
                        for kt in range(NKT):
                            exps = emit_qkexp(pi, g, qc0, kt)
                            if prev is not None:
                                ppi, pg, pqc0, pkt, pexps = prev
                                emit_pv(ppi, pg, pqc0, pkt, pexps)
                                if pkt == NKT - 1:
                                    emit_norm(ppi, pg, pqc0)
                                    for fn in between.get(ppi, ()):
                                        fn()
                            prev = (pi, g, qc0, kt, exps)
                    ppi, pg, pqc0, pkt, pexps = prev
                    emit_pv(ppi, pg, pqc0, pkt, pexps)
                    emit_norm(ppi, pg, pqc0)

                return run

            # ---------------- pipelined emission ----------------
            with (
                tc.tile_pool(name="xtp", bufs=8) as xtpool,
                tc.tile_pool(name="vchp", bufs=3) as vchpool,
                tc.tile_pool(name="ps1", bufs=2, space="PSUM") as ps1,
                tc.tile_pool(name="ps1v", bufs=1, space="PSUM") as ps1v,
                tc.tile_pool(name="ps2", bufs=2, space="PSUM") as ps2,
                tc.tile_pool(name="psrep", bufs=1, space="PSUM") as reppool,
                tc.tile_pool(name="psswp", bufs=1, space="PSUM") as swppool,
            ):
                statps = {}
                for tname in ("q", "k"):
                    statps[(tname, 0)] = ps2.tile(
                        [40, 512], F32, tag="stat", name=f"stat_{tname}0")
                for r in range(4):
                    emit_proj_row(r, xtpool, vchpool, ps1, ps1v, statps)
                emit_statmath("q", 0, statps)
                emit_statmath("k", 0, statps)
                # batch-0 LN interleaved with batch-1 projections
                for tname in ("q", "k"):
                    statps[(tname, 1)] = ps2.tile(
                        [40, 512], F32, tag="stat", name=f"stat_{tname}1")
                for jj in range(4):
                    xts = emit_proj_load(4 + jj, xtpool)
                    cosc, sinc = emit_cs_load(jj)
                    tn_q = emit_apply_pe_head("q", jj, 0, lnbq_sb, reppool)
                    emit_proj_m(4 + jj, 0, xts, vchpool, ps1, ps1v, statps)
                    emit_apply_pe_tail("q", jj, tn_q, cosc, sinc, swppool)
                    emit_proj_m(4 + jj, 1, xts, vchpool, ps1, ps1v, statps)
                    tn_k = emit_apply_pe_head("k", jj, 1, lnbk_sb, reppool)
                    emit_proj_m(4 + jj, 2, xts, vchpool, ps1, ps1v, statps)
                    emit_apply_pe_tail("k", jj, tn_k, cosc, sinc, swppool)
                emit_statmath("q", 1, statps)
                emit_statmath("k", 1, statps)

            # deferred constant loads (keep startup queues clear)
            nc.sync.dma_start(out=wo_sb[:], in_=wo_d.ap()[:, :, :])
            nc.sync.dma_start(out=borep_sb[:], in_=borep_d.ap()[:, :])

            # AllToAll buffers (staged per-pair inside the SDPA stream)
            a2a_in = dpool.tile([NCORE, 128, 512], BF16)
            a2a_out = dpool.tile([NCORE, 128, 512], BF16)

            # batch-1 LN interleaved with batch-0 SDPA
            with (
                tc.tile_pool(name="expp", bufs=4) as exppool,
                tc.tile_pool(name="sp3", bufs=2) as sp3,
                tc.tile_pool(name="sp4", bufs=2) as sp4,
                tc.tile_pool(name="ps_sc", bufs=2, space="PSUM") as ps_sc,
                tc.tile_pool(name="ps_ctx", bufs=4, space="PSUM") as ps_ctx,
            ):
                emit_apply_gp("q", 0, lnwq_sb, lnbq_sb)
                emit_apply_gp("k", 0, lnwk_sb, lnbk_sb)
                emit_apply_gp("q", 1, lnwq_sb, lnbq_sb)
                run_sdpa = make_sdpa(exppool, sp3, sp4, ps_sc, ps_ctx,
                                     a2a_in)
                run_sdpa(
                    [(0, 0), (0, 2), (1, 0), (1, 2)],
                    {
                        0: (lambda: emit_apply_gp("k", 1, lnwk_sb, lnbk_sb),
                            lambda: emit_apply_gp("q", 2, lnwq_sb, lnbq_sb),
                            lambda: emit_apply_gp("k", 2, lnwk_sb, lnbk_sb)),
                        1: (lambda: emit_apply_gp("q", 3, lnwq_sb, lnbq_sb),
                            lambda: emit_apply_gp("k", 3, lnwk_sb, lnbk_sb)),
                    })

            # ================= AllToAll (bf16) =================
            nc.gpsimd.collective_compute(
                "AllToAll", mybir.AluOpType.bypass,
                ins=[a2a_in.opt()], outs=[a2a_out.opt()],
                replica_groups=[list(range(NCORE))],
            )

            # ================= output projection (bf16) ==============
            with (
                tc.tile_pool(name="wop", bufs=3) as wopool,
                tc.tile_pool(name="sp5", bufs=4) as sp5,
                tc.tile_pool(name="ps_out", bufs=4, space="PSUM") as ps_out,
            ):
                ops = [ps_out.tile([128, 1024], F32, tag="outp",
                                   name=f"outp{i}") for i in range(4)]
                for kt in range(KT_DIM):
                    cg = wopool.tile([128, 512], BF16, tag="ctxg",
                                     name=f"cg{kt}")
                    nc.sync.dma_start(out=cg[:], in_=a2a_out[kt])
                    for rt in range(4):
                        for nh in range(2):
                            nc.tensor.matmul(
                                ops[rt][:, nh * 512:(nh + 1) * 512],
                                cg[:, rt * 128:(rt + 1) * 128],
                                wo_sb[:, kt, nh * 512:(nh + 1) * 512],
                                start=(kt == 0), stop=(kt == KT_DIM - 1))
                for rt in range(4):
                    osb = sp5.tile([128, DIM], F32, tag="osb",
                                   name=f"osb{rt}")
                    nc.vector.tensor_tensor(osb[:], ops[rt][:], borep_sb[:],
                                            ADD)
                    nc.sync.dma_start(
                        out=out_d.ap()[rt * 128:(rt + 1) * 128, :],
                        in_=osb[:])

            if DEBUG_OUTPUTS:
                for g in range(B):
                    nc.sync.dma_start(
                        out=dbg_qrot.ap()[:, g * N:(g + 1) * N],
                        in_=qkt[("q", g)][:].bitcast(F32))
                    nc.sync.dma_start(
                        out=dbg_krot.ap()[:, g * N:(g + 1) * N],
                        in_=qkt[("k", g)][:].bitcast(F32))
                nc.gpsimd.dma_start(out=dbg_ctxn.ap()[0:64, :],
                                    in_=ctxn_a[:])
                nc.gpsimd.dma_start(out=dbg_ctxn.ap()[64:128, :],
                                    in_=ctxn_b[:])

    nc.compile()
    return nc


# ---------------------------------------------------------------- host side
def prepare_in_maps(x, rotary_cos, rotary_sin, Wq, bq, Wk, bk, Wv, bv,
                    q_norm_w, q_norm_b, k_norm_w, k_norm_b, Wo, bo):
    import ml_dtypes

    x = np.asarray(x, np.float32)
    xT = _round_fp32r(np.ascontiguousarray(x.reshape(R, DIM).T))

    Wcat = np.concatenate([np.asarray(Wq, np.float32),
                           np.asarray(Wk, np.float32),
                           np.asarray(Wv, np.float32)], axis=1)
    bcat = np.concatenate([np.asarray(bq, np.float32),
                           np.asarray(bk, np.float32),
                           np.asarray(bv, np.float32)])

    def head_cols(h, part):
        s = 192 * h + 64 * part
        return np.arange(s, s + 64)

    cos_flat = np.asarray(rotary_cos, np.float32).reshape(R, HD).T
    sin_flat = np.asarray(rotary_sin, np.float32).reshape(R, HD).T
    sinm = sin_flat.copy()
    sinm[0:32] = -sin_flat[0:32]
    cos_rep = np.ascontiguousarray(np.tile(cos_flat, (2, 1)))
    sinm_rep = np.ascontiguousarray(np.tile(sinm, (2, 1)))

    # stats lhsT, pre-permuted to [128, RC, 2, 40]
    onesblk = np.zeros((128, RC, 2, 40), np.float32)
    for j in range(RC):
        jj = j % 4
        onesblk[0:64, j, 0, 2 * jj] = 1.0
        onesblk[64:128, j, 0, 2 * jj + 1] = 1.0
        onesblk[0:64, j, 1, 32 + 2 * jj] = 1.0
        onesblk[64:128, j, 1, 32 + 2 * jj + 1] = 1.0

    # LN biases [2, 128, 1] and weight-masked broadcast lhsT [2, 2, 128]
    lnb = np.stack([
        np.tile(np.asarray(q_norm_b, np.float32), 2)[:, None],
        np.tile(np.asarray(k_norm_b, np.float32), 2)[:, None],
    ])
    lnw = np.stack([
        np.tile(np.asarray(q_norm_w, np.float32), 2)[:, None],
        np.tile(np.asarray(k_norm_w, np.float32), 2)[:, None],
    ])
    wbc = np.zeros((8, 2, 4, 128), np.float32)
    for i, w in enumerate((q_norm_w, k_norm_w)):
        wt = np.tile(np.asarray(w, np.float32), 2)
        for jj in range(4):
            wbc[2 * jj, i, jj, 0:64] = wt[0:64]
            wbc[2 * jj + 1, i, jj, 64:128] = wt[64:128]
    wbc = _round_fp32r(wbc)

    # RoPE half-swap permutation: out partition p reads src perm_src[p]
    perm = np.zeros((128, 128), np.float32)
    for p in range(128):
        src = p + 32 if (p % 64) < 32 else p - 32
        perm[src, p] = 1.0
    perm = _round_fp32r(perm)

    ident = np.eye(128, dtype=np.float32)
    ones64 = np.ones((128, 4 * NKT), np.float32)
    borep = np.tile(np.asarray(bo, np.float32)[None, :], (128, 1))
    # Wo pre-permuted to [128, KT_DIM, DIM] bf16
    wo_r = np.ascontiguousarray(
        np.asarray(Wo, np.float32).reshape(KT_DIM, 128, DIM)
        .transpose(1, 0, 2)).astype(ml_dtypes.bfloat16)

    in_maps = []
    for c in range(NCORE):
        hA, hB = 2 * c, 2 * c + 1
        cols = np.concatenate([
            head_cols(hA, 0), head_cols(hB, 0),
            head_cols(hA, 1), head_cols(hB, 1),
            head_cols(hA, 2), head_cols(hB, 2),
        ])
        wqkv_c = _round_fp32r(np.ascontiguousarray(
            Wcat[:, cols].reshape(KT_DIM, 128, 384).transpose(1, 0, 2)))
        bqkv_c = np.ascontiguousarray(bcat[cols].reshape(3, 128, 1))
        in_maps.append({
            "xT": xT,
            "wqkv": wqkv_c,
            "bqkv": bqkv_c,
            "onesblk": onesblk,
            "lnb": lnb,
            "lnw": lnw,
            "wbc": wbc,
            "perm": perm,
            "cosr": cos_rep,
            "sinm": sinm_rep,
            "ident": ident,
            "ones64": ones64,
            "wo": wo_r,
            "borep": borep,
        })
    return in_maps


def assemble_output(results):
    out = np.empty((R, DIM), np.float32)
    for c in range(NCORE):
        out[c * 512:(c + 1) * 512] = results[c]["out"]
    return out.reshape(B, N, DIM)


_NC_CACHE = []


def kernel(**inputs) -> np.ndarray:
    if not _NC_CACHE:
        _NC_CACHE.append(build())
    nc = _NC_CACHE[0]
    in_maps = prepare_in_maps(**inputs)
    res = run_bass_kernel_spmd(nc, in_maps, core_ids=list(range(NCORE)))
    return assemble_output(res.results)


# revision 25
# speedup vs baseline: 1.0108x; 1.0108x over previous
"""Trainium2 Bass kernel for nn_Attention_17008070493108.

Dense transformer attention block: QKV proj -> per-head LayerNorm -> RoPE
-> SDPA -> out proj, for x[2, 2048, 1024], H=16 heads, head_dim=64.

Sharding: tensor-parallel over heads. Each of the 8 NeuronCores owns 2
heads end-to-end (QKV column slices, norm, RoPE, attention). The
per-head context vectors are exchanged with a single AllToAll so each
core finishes the output projection (contraction over the full 1024
model dims) for its own 512-row slice of the output; the host
concatenates row slices.

Layouts (per core):
  xT          [1024, 4096] model-dim on partitions (host-transposed x)
  QT/KT       [128, 2048]x2 (batch-split) heads stacked on partitions
  scoresT     [128 keys, q] key tiles on partitions; softmax denominator
                          via a ones-column appended to V (ctx_aug row 64)
  ctx         [65, 512] psum x4 -> normalize -> ctxn [128, 4096] bf16
              -> AllToAll -> out rows [512, 1024]

Fast paths vs the original emission:
  * LayerNorm stats -> Sqrt + reciprocal_approx_fast (no slow DVE
    reciprocal), kept in SBUF for batch 0; the per-column rstd and
    mu*rstd broadcasts for the batch-0 LN apply are contraction-8 PE
    matmuls (lhsT = per-chunk LN-weight-masked selectors) so the LN
    weight is folded in for free, and (x*wr + b - w*mu*r) is a single
    fused scalar_tensor_tensor op. The RoPE half-swap is a 128x128
    permutation matmul. These apply stages are interleaved with the
    batch-1 projection matmul groups so their DVE latency hides under
    PE work.
  * SDPA is globally software-pipelined: QK+exp of kt-step i+1 are
    emitted before PV of step i, across qc-pair boundaries, so the
    Exp unit (the SDPA bottleneck, ~141us of Activation time) is never
    stalled by the in-order PE pipe finishing a PV group.
  * Softmax normalize copies psum ctx to SBUF f32, derives 1/den via
    reciprocal_approx_fast (no DRAM roundtrips), and stages each
    finished AllToAll chunk immediately so every core reaches the
    collective as early as possible.
  * Wo/borep loads are deferred past the startup burst; wqkv is split
    q|kv so the first projection matmul starts ~15us earlier; host
    pre-permutes wqkv/onesblk/wo so constant DMAs are one descriptor
    per partition.
Batch-1 LN applies (which overlap SDPA, where all 8 PSUM banks are
busy) keep the gpsimd partition_broadcast path.
"""

import numpy as np

from concourse import bacc, tile, mybir
from concourse.bass_utils import run_bass_kernel_spmd

# ---------------------------------------------------------------- constants
DIM = 1024
H = 16
HD = 64
B = 2
N = 2048
R = B * N          # 4096 flattened rows
NCORE = 8
EPS = 1e-6

F32 = mybir.dt.float32
F32R = mybir.dt.float32r
BF16 = mybir.dt.bfloat16
ADD = mybir.AluOpType.add
SUB = mybir.AluOpType.subtract
MUL = mybir.AluOpType.mult

RC = R // 512        # 8 row chunks of 512
KT_DIM = DIM // 128  # 8 contraction tiles for the projections
NQC = N // 512       # 4 q chunks per batch
NKT = N // 128       # 16 key tiles per batch
VSTRIDE = 130        # per-keytile V_aug block: [vA(64) | 1 | vB(64) | 1]

DEBUG_OUTPUTS = False


def _round_fp32r(x: np.ndarray) -> np.ndarray:
    """Round fp32 to fp32r (11-bit mantissa, RNE)."""
    u = np.ascontiguousarray(x, dtype=np.float32).view(np.uint32)
    lsb = (u >> np.uint32(12)) & np.uint32(1)
    r = (u + np.uint32(0x7FF) + lsb) & np.uint32(0xFFFFF000)
    return r.view(np.float32)


# ---------------------------------------------------------------- graph
def build():
    nc = bacc.Bacc("TRN2", target_bir_lowering=False, debug=False,
                   num_devices=NCORE)

    # ---- DRAM parameters (host pre-permuted for contiguous DMA)
    xT_d = nc.dram_tensor("xT", [DIM, R], F32R, kind="ExternalInput")
    wqkv_d = nc.dram_tensor("wqkv", [128, KT_DIM, 384], F32R,
                            kind="ExternalInput")
    bqkv_d = nc.dram_tensor("bqkv", [3, 128, 1], F32, kind="ExternalInput")
    # stats lhsT: [:, :, 0, c] x-sums col {2jj+h}, [:, :, 1, c] sq-sums
    # col {32+2jj+h} (offset 32 keeps DVE reads partition-aligned); both
    # accumulate into one [40, 512] psum bank.
    onesblk_d = nc.dram_tensor("onesblk", [128, RC, 2, 40], F32R,
                               kind="ExternalInput")
    lnb_d = nc.dram_tensor("lnb", [2, 128, 1], F32, kind="ExternalInput")
    lnw_d = nc.dram_tensor("lnw", [2, 128, 1], F32, kind="ExternalInput")
    wbc_d = nc.dram_tensor("wbc", [8, 2, 4, 128], F32R,
                           kind="ExternalInput")
    perm_d = nc.dram_tensor("perm", [128, 128], F32R, kind="ExternalInput")
    cos_d = nc.dram_tensor("cosr", [128, R], F32, kind="ExternalInput")
    sinm_d = nc.dram_tensor("sinm", [128, R], F32, kind="ExternalInput")
    ident_d = nc.dram_tensor("ident", [128, 128], F32, kind="ExternalInput")
    ones_d = nc.dram_tensor("ones64", [128, 4 * NKT], F32R,
                            kind="ExternalInput")
    wo_d = nc.dram_tensor("wo", [128, KT_DIM, DIM], BF16,
                          kind="ExternalInput")
    borep_d = nc.dram_tensor("borep", [128, DIM], F32, kind="ExternalInput")
    out_d = nc.dram_tensor("out", [R // NCORE, DIM], F32, kind="ExternalOutput")
    if DEBUG_OUTPUTS:
        dbg_qrot = nc.dram_tensor("dbg_qrot", [128, R], F32,
                                  kind="ExternalOutput")
        dbg_krot = nc.dram_tensor("dbg_krot", [128, R], F32,
                                  kind="ExternalOutput")
        dbg_ctxn = nc.dram_tensor("dbg_ctxn", [128, R], BF16,
                                  kind="ExternalOutput")

    with tile.TileContext(nc) as tc:
        with (
            tc.tile_pool(name="const", bufs=1) as cpool,
            tc.tile_pool(name="persist", bufs=1) as ppool,
            tc.tile_pool(name="chp", bufs=2) as chpool,
            tc.tile_pool(name="statp", bufs=6) as statpool,
            tc.tile_pool(name="gpp", bufs=1) as gppool,
            tc.tile_pool(name="stagp", bufs=4) as stagpool,
            tc.tile_pool(name="dram", bufs=1, space="DRAM") as dpool,
        ):
            # ---- constants in SBUF (contiguous per-partition DMAs)
            # q/k weight columns first so row-0 projection starts ASAP
            wqkv_sb = cpool.tile([128, KT_DIM, 384], F32R)
            nc.sync.dma_start(out=wqkv_sb[:, :, 0:128],
                              in_=wqkv_d.ap()[:, :, 0:128])
            bq_sb = cpool.tile([128, 1], F32)
            bk_sb = cpool.tile([128, 1], F32)
            bv_sb = cpool.tile([128, 1], F32)
            nc.sync.dma_start(out=bq_sb[:], in_=bqkv_d.ap()[0])
            nc.sync.dma_start(out=bk_sb[:], in_=bqkv_d.ap()[1])
            nc.sync.dma_start(out=bv_sb[:], in_=bqkv_d.ap()[2])
            onesblk_sb = cpool.tile([128, RC, 2, 40], F32R)
            nc.sync.dma_start(out=onesblk_sb[:], in_=onesblk_d.ap()[:, :, :, :])
            nc.sync.dma_start(out=wqkv_sb[:, :, 128:384],
                              in_=wqkv_d.ap()[:, :, 128:384])
            lnbq_sb = cpool.tile([128, 1], F32)
            lnbk_sb = cpool.tile([128, 1], F32)
            nc.sync.dma_start(out=lnbq_sb[:], in_=lnb_d.ap()[0])
            nc.sync.dma_start(out=lnbk_sb[:], in_=lnb_d.ap()[1])
            lnwq_sb = cpool.tile([128, 1], F32)
            lnwk_sb = cpool.tile([128, 1], F32)
            nc.sync.dma_start(out=lnwq_sb[:], in_=lnw_d.ap()[0])
            nc.sync.dma_start(out=lnwk_sb[:], in_=lnw_d.ap()[1])
            wbc_sb = cpool.tile([8, 2, 4, 128], F32R)
            nc.sync.dma_start(out=wbc_sb[:], in_=wbc_d.ap()[:, :, :, :])
            perm_sb = cpool.tile([128, 128], F32R)
            nc.sync.dma_start(out=perm_sb[:], in_=perm_d.ap()[:, :])
            ident_sb = cpool.tile([128, 128], F32)
            nc.sync.dma_start(out=ident_sb[:], in_=ident_d.ap()[:, :])
            borep_sb = cpool.tile([128, DIM], F32)
            wo_sb = cpool.tile([128, KT_DIM, DIM], BF16)

            # ---- persistent tensors (batch-split Q/K; in-place LN+RoPE)
            qkt = {}
            for g in range(B):
                qkt[("q", g)] = ppool.tile([128, N], F32R, tag=f"q{g}",
                                           name=f"qraw{g}")
                qkt[("k", g)] = ppool.tile([128, N], F32R, tag=f"k{g}",
                                           name=f"kraw{g}")
            vaug = ppool.tile([128, 2 * NKT * VSTRIDE], F32R, tag="vaug")
            ctxn_a = ppool.tile([64, R], BF16, tag="ctxn_a")
            ctxn_b = ppool.tile([64, R], BF16, tag="ctxn_b")
            # batch-0 stats kept in SBUF for the PE-broadcast LN path
            stat_sb = {}
            for tname in ("q", "k"):
                stat_sb[(tname, "r")] = ppool.tile(
                    [8, 512], F32R, tag=f"rstd_{tname}",
                    name=f"rstd_{tname}")
                stat_sb[(tname, "m")] = ppool.tile(
                    [8, 512], F32R, tag=f"mur_{tname}", name=f"mur_{tname}")

            nc.gpsimd.dma_start(
                out=vaug[:].rearrange("p (k c) -> p k c", c=65)[:, :, 64:65],
                in_=ones_d.ap()[:, :])

            stat_dr = {}

            # ---------------- emission helpers ----------------
            def emit_proj_load(r, xtpool):
                """DMA the 8 xT contraction tiles for row-chunk r."""
                xts = []
                for kt in range(KT_DIM):
                    xt = xtpool.tile([128, 512], F32R, tag="xt",
                                     name=f"xt_{r}_{kt}")
                    nc.sync.dma_start(
                        out=xt[:],
                        in_=xT_d.ap()[kt * 128:(kt + 1) * 128,
                                      r * 512:(r + 1) * 512])
                    xts.append(xt)
                return xts

            def emit_proj_m(r, m, xts, vchpool, ps1, ps1v, statps):
                """Project row-chunk r for one of q/k/v (+ stats MMs)."""
                g, jj = r // 4, r % 4
                name, bias = (("q", bq_sb), ("k", bk_sb),
                              ("v", bv_sb))[m]
                ps = ps1.tile([128, 512], F32, tag="proj",
                              name=f"proj_{m}_{r}")
                for kt in range(KT_DIM):
                    nc.tensor.matmul(
                        ps[:], wqkv_sb[:, kt, m * 128:(m + 1) * 128],
                        xts[kt][:],
                        start=(kt == 0), stop=(kt == KT_DIM - 1))
                if m < 2:
                    dest = qkt[(name, g)]
                    nc.vector.tensor_scalar(
                        dest[:, jj * 512:(jj + 1) * 512], ps[:],
                        bias[:], None, ADD)
                    sps = statps[(name, g)]
                    nc.tensor.matmul(
                        sps[:], onesblk_sb[:, r, 0, :],
                        dest[:, jj * 512:(jj + 1) * 512],
                        start=(jj == 0), stop=False)
                    sqc = chpool.tile([128, 512], F32R, tag="sqc",
                                      name=f"sqc_{name}_{r}")
                    nc.scalar.square(
                        sqc[:],
                        dest[:, jj * 512:(jj + 1) * 512].bitcast(F32))
                    nc.tensor.matmul(
                        sps[:], onesblk_sb[:, r, 1, :], sqc[:],
                        start=False, stop=(jj == 3))
                else:
                    vch = vchpool.tile([128, 512], F32, tag="vch",
                                       name=f"vch_{r}")
                    nc.scalar.add(vch[:], ps[:], bias[:])
                    for sseg in range(4):
                        kt_glob = r * 4 + sseg
                        tps = ps1v.tile([128, 128], F32, tag="vtr",
                                        name=f"vtr_{kt_glob}")
                        nc.tensor.transpose(
                            tps[:], vch[:, sseg * 128:(sseg + 1) * 128],
                            ident_sb[:])
                        vb = kt_glob * VSTRIDE
                        nc.scalar.copy(
                            vaug[:, vb:vb + 64], tps[:, 0:64])
                        nc.scalar.copy(
                            vaug[:, vb + 65:vb + 129], tps[:, 64:128])

            def emit_proj_row(r, xtpool, vchpool, ps1, ps1v, statps):
                xts = emit_proj_load(r, xtpool)
                for m in range(3):
                    emit_proj_m(r, m, xts, vchpool, ps1, ps1v, statps)

            def emit_statmath(name, g, statps):
                """stat bank [40, 512]: rows 0-7 x-sums, 32-39 sq-sums.
                g=0: rstd/mu*rstd -> persistent SBUF (PE-broadcast path).
                g=1: staged to DRAM for the gpsimd-broadcast path."""
                sps = statps[(name, g)]
                mu = statpool.tile([8, 512], F32, tag="stat_sb",
                                   name=f"mu_{name}{g}")
                msqe = statpool.tile([8, 512], F32, tag="stat_sb",
                                     name=f"msqe_{name}{g}")
                nc.vector.tensor_scalar(mu[:], sps[0:8, :], 1.0 / HD,
                                        None, MUL)
                nc.vector.tensor_scalar(msqe[:], sps[32:40, :], 1.0 / HD,
                                        EPS, MUL, ADD)
                var = statpool.tile([8, 512], F32, tag="stat_sb",
                                    name=f"var_{name}{g}")
                nc.vector.tensor_tensor(var[:], mu[:], mu[:], MUL)
                nc.vector.tensor_tensor(var[:], msqe[:], var[:], SUB)
                sd = statpool.tile([8, 512], F32, tag="stat_sb",
                                   name=f"sd_{name}{g}")
                nc.scalar.activation(sd[:], var[:],
                                     mybir.ActivationFunctionType.Sqrt)
                if g == 0:
                    rtmp = statpool.tile([8, 512], F32, tag="stat_sb",
                                         name=f"rtmp_{name}{g}")
                    nc.vector.reciprocal_approx_fast(rtmp[:], sd[:])
                    # DVE writes to F32R tiles round properly for the
                    # downstream fp32r rep matmuls
                    nc.vector.tensor_copy(stat_sb[(name, "r")][:], rtmp[:])
                    nc.vector.tensor_tensor(stat_sb[(name, "m")][:], mu[:],
                                            rtmp[:], MUL)
                else:
                    rstd = statpool.tile([8, 512], F32, tag="stat_sb",
                                         name=f"rstd_{name}{g}")
                    nc.vector.reciprocal_approx_fast(rstd[:], sd[:])
                    murstd = statpool.tile([8, 512], F32, tag="stat_sb",
                                           name=f"murstd_{name}{g}")
                    nc.vector.tensor_tensor(murstd[:], mu[:],
                                            rstd[:], MUL)
                    rdr = dpool.tile([8, 512], F32, name=f"rstd_dr_{name}{g}")
                    mdr = dpool.tile([8, 512], F32, name=f"mur_dr_{name}{g}")
                    nc.sync.dma_start(out=rdr[:], in_=rstd[:])
                    nc.sync.dma_start(out=mdr[:], in_=murstd[:])
                    stat_dr[(name, g)] = (rdr, mdr)

            def emit_cs_load(jj):
                """cos/sin for batch-0 chunk jj (shared by q and k)."""
                jsl = slice(jj * 512, (jj + 1) * 512)
                cosc = chpool.tile([128, 512], F32, tag="cosc",
                                   name=f"cosc_0{jj}")
                sinc = chpool.tile([128, 512], F32, tag="sinc",
                                   name=f"sinc_0{jj}")
                nc.sync.dma_start(out=cosc[:], in_=cos_d.ap()[:, jsl])
                nc.sync.dma_start(out=sinc[:], in_=sinm_d.ap()[:, jsl])
                return cosc, sinc

            def emit_apply_pe_head(name, jj, wi, b_sb, reppool):
                """LN apply for batch-0 chunk jj: rep matmuls + fused DVE."""
                traw = qkt[(name, 0)]
                rstd = stat_sb[(name, "r")]
                murstd = stat_sb[(name, "m")]
                jsl = slice(jj * 512, (jj + 1) * 512)
                rep = reppool.tile([128, 1024], F32, tag="rep",
                                   name=f"rep_{name}_0{jj}")
                nc.tensor.matmul(rep[:, 0:512], wbc_sb[:, wi, jj, :],
                                 rstd[:], start=True, stop=True)
                nc.tensor.matmul(rep[:, 512:1024], wbc_sb[:, wi, jj, :],
                                 murstd[:], start=True, stop=True)
                tn = chpool.tile([128, 512], F32R, tag="tn",
                                 name=f"tn_{name}_0{jj}")
                nc.vector.tensor_tensor(tn[:], traw[:, jsl].bitcast(F32),
                                        rep[:, 0:512], MUL)
                nc.vector.scalar_tensor_tensor(
                    tn[:], tn[:].bitcast(F32), b_sb[:], rep[:, 512:1024],
                    ADD, SUB)
                return tn

            def emit_apply_pe_tail(name, jj, tn, cosc, sinc, swppool):
                """RoPE for batch-0 chunk jj: perm matmul + rotate-combine."""
                traw = qkt[(name, 0)]
                jsl = slice(jj * 512, (jj + 1) * 512)
                swp = swppool.tile([128, 512], F32, tag="swp",
                                   name=f"swp_{name}_0{jj}")
                nc.tensor.matmul(swp[:], perm_sb[:], tn[:],
                                 start=True, stop=True)
                t1 = chpool.tile([128, 512], F32, tag="t1",
                                 name=f"t1_{name}_0{jj}")
                nc.gpsimd.tensor_tensor(t1[:], tn[:].bitcast(F32), cosc[:],
                                        MUL)
                s2 = chpool.tile([128, 512], F32, tag="t1",
                                 name=f"s2_{name}_0{jj}")
                nc.vector.tensor_tensor(s2[:], swp[:], sinc[:], MUL)
                nc.vector.tensor_tensor(traw[:, jsl], t1[:], s2[:], ADD)

            def emit_apply_gp(name, jj, w_sb, b_sb):
                """Batch-1 LN apply + RoPE (gpsimd broadcasts, no PSUM;
                runs overlapped with SDPA where all 8 banks are taken)."""
                g = 1
                traw = qkt[(name, g)]
                rdr, mdr = stat_dr[(name, g)]
                jsl = slice(jj * 512, (jj + 1) * 512)
                gsl = slice(g * N + jj * 512, g * N + (jj + 1) * 512)
                cosc = chpool.tile([128, 512], F32, tag="cosc",
                                   name=f"cosc_{name}_{g}{jj}")
                sinc = chpool.tile([128, 512], F32, tag="sinc",
                                   name=f"sinc_{name}_{g}{jj}")
                nc.sync.dma_start(out=cosc[:], in_=cos_d.ap()[:, gsl])
                nc.sync.dma_start(out=sinc[:], in_=sinm_d.ap()[:, gsl])
                rep_r = gppool.tile([128, 512], F32, tag="rep_r",
                                    name=f"rep_r_{name}_{g}{jj}")
                rep_m = gppool.tile([128, 512], F32, tag="rep_m",
                                    name=f"rep_m_{name}_{g}{jj}")
                for h in range(2):
                    stg_r = stagpool.tile([1, 512], F32, tag="stg",
                                          name=f"sr_{name}_{g}{jj}_{h}")
                    stg_m = stagpool.tile([1, 512], F32, tag="stg",
                                          name=f"sm_{name}_{g}{jj}_{h}")
                    nc.sync.dma_start(out=stg_r[:], in_=rdr[2 * jj + h])
                    nc.sync.dma_start(out=stg_m[:], in_=mdr[2 * jj + h])
                    if h == 0:
                        nc.gpsimd.partition_broadcast(
                            rep_r[0:64, :], stg_r[:], channels=64)
                        nc.gpsimd.partition_broadcast(
                            rep_m[0:64, :], stg_m[:], channels=64)
                    else:
                        tmp_r = stagpool.tile([64, 512], F32, tag="tmpb",
                                              name=f"tr_{name}_{g}{jj}")
                        tmp_m = stagpool.tile([64, 512], F32, tag="tmpb",
                                              name=f"tm_{name}_{g}{jj}")
                        nc.gpsimd.partition_broadcast(
                            tmp_r[:], stg_r[:], channels=64)
                        nc.gpsimd.partition_broadcast(
                            tmp_m[:], stg_m[:], channels=64)
                        nc.sync.dma_start(out=rep_r[64:128, :],
                                          in_=tmp_r[:])
                        nc.sync.dma_start(out=rep_m[64:128, :],
                                          in_=tmp_m[:])
                tn = chpool.tile([128, 512], F32, tag="tn",
                                 name=f"tn_{name}_{g}{jj}")
                nc.vector.tensor_tensor(tn[:], traw[:, jsl].bitcast(F32),
                                        rep_r[:], MUL)
                nc.vector.tensor_tensor(tn[:], tn[:], rep_m[:], SUB)
                nc.vector.tensor_scalar(tn[:], tn[:], w_sb[:], b_sb[:],
                                        MUL, ADD)
                swp = gppool.tile([128, 512], F32, tag="swp",
                                  name=f"swp_{name}_{g}{jj}")
                for (dst, src) in ((0, 32), (32, 0), (64, 96), (96, 64)):
                    nc.sync.dma_start(out=swp[dst:dst + 32, :],
                                      in_=tn[src:src + 32, :])
                t1 = chpool.tile([128, 512], F32, tag="t1",
                                 name=f"t1_{name}_{g}{jj}")
                nc.vector.tensor_tensor(t1[:], tn[:], cosc[:], MUL)
                nc.vector.tensor_tensor(swp[:], swp[:], sinc[:], MUL)
                nc.vector.tensor_tensor(traw[:, jsl], t1[:], swp[:], ADD)

            def make_sdpa(exppool, sp3, sp4, ps_sc, ps_ctx, a2a_in):
                """Globally software-pipelined SDPA over all qc-pairs:
                QK/exp run one kt-step ahead of PV across pair boundaries
                so the Exp unit never waits for the in-order PE pipe; each
                pair's normalize + AllToAll staging happens as soon as its
                last PV retires."""
                ctxmap = {}

                def emit_qkexp(pi, g, qc0, kt):
                    qrot = qkt[("q", g)]
                    krot = qkt[("k", g)]
                    ksl = slice(kt * 128, (kt + 1) * 128)
                    scs = {}
                    for qc in (qc0, qc0 + 1):
                        scs[qc] = ps_sc.tile([128, 1024], F32, tag="sc",
                                             name=f"sc_{g}{qc}{kt}")
                    for h, psl in ((0, slice(0, 64)), (1, slice(64, 128))):
                        for qc in (qc0, qc0 + 1):
                            qsl = slice(qc * 512, (qc + 1) * 512)
                            nc.tensor.matmul(
                                scs[qc][:, h * 512:(h + 1) * 512],
                                krot[psl, ksl], qrot[psl, qsl],
                                start=True, stop=True,
                                tile_position=(h * 64, 0))
                    exps = {}
                    for qc in (qc0, qc0 + 1):
                        expt = exppool.tile([128, 1024], F32R, tag="expt",
                                            name=f"ex_{g}{qc}{kt}")
                        nc.scalar.activation(
                            expt[:], scs[qc][:],
                            mybir.ActivationFunctionType.Exp,
                            scale=float(HD) ** -0.5)
                        exps[qc] = expt
                    return exps

                def emit_pv(pi, g, qc0, kt, exps):
                    if kt == 0:
                        d = {}
                        for qc in (qc0, qc0 + 1):
                            for h in range(2):
                                d[(qc, h)] = ps_ctx.tile(
                                    [65, 512], F32, tag="ctx",
                                    name=f"ctx_{g}{qc}{h}")
                        ctxmap[pi] = d
                    ctxps = ctxmap[pi]
                    vbase = (g * NKT + kt) * VSTRIDE
                    for h in range(2):
                        vsl = slice(vbase + h * 65, vbase + (h + 1) * 65)
                        for qc in (qc0, qc0 + 1):
                            nc.tensor.matmul(
                                ctxps[(qc, h)][:], vaug[:, vsl],
                                exps[qc][:, h * 512:(h + 1) * 512],
                                start=(kt == 0), stop=(kt == NKT - 1))

                def emit_norm(pi, g, qc0):
                    ctxps = ctxmap.pop(pi)
                    for qc in (qc0, qc0 + 1):
                        qsl = slice(g * N + qc * 512,
                                    g * N + (qc + 1) * 512)
                        for h, dst in ((0, ctxn_a), (1, ctxn_b)):
                            cu = sp4.tile([65, 512], F32, tag="cu",
                                          name=f"cu{g}{qc}{h}")
                            nc.vector.tensor_copy(cu[:], ctxps[(qc, h)][:])
                            dn = sp3.tile([1, 512], F32, tag="dn",
                                          name=f"dn{g}{qc}{h}")
                            nc.sync.dma_start(out=dn[:], in_=cu[64:65, :])
                            rc = sp3.tile([1, 512], F32, tag="rc",
                                          name=f"rc{g}{qc}{h}")
                            nc.vector.reciprocal_approx_fast(rc[:], dn[:])
                            rep = sp3.tile([64, 512], F32, tag="rep",
                                           name=f"rp{g}{qc}{h}")
                            nc.gpsimd.partition_broadcast(rep[:], rc[:],
                                                          channels=64)
                            nc.vector.tensor_tensor(
                                dst[:, qsl], cu[0:64, :], rep[:], MUL)
                        # stage this dest chunk for the AllToAll now
                        j = g * 4 + qc
                        nc.gpsimd.dma_start(
                            out=a2a_in[j][0:64, :],
                            in_=ctxn_a[:, g * N + qc * 512:
                                       g * N + (qc + 1) * 512])
                        nc.gpsimd.dma_start(
                            out=a2a_in[j][64:128, :],
                            in_=ctxn_b[:, g * N + qc * 512:
                                       g * N + (qc + 1) * 512])

                def run(pairs, between):
                    prev = None
                    for pi, (g, qc0) in enumerate(pairs):
                        for kt in range(NKT):
                            exps = emit_qkexp(pi, g, qc0, kt)
                            if prev is not None:
                                ppi, pg, pqc0, pkt, pexps = prev
                                emit_pv(ppi, pg, pqc0, pkt, pexps)
                                if pkt == NKT - 1:
                                    emit_norm(ppi, pg, pqc0)
                                    for fn in between.get(ppi, ()):
                                        fn()
                            prev = (pi, g, qc0, kt, exps)
                    ppi, pg, pqc0, pkt, pexps = prev
                    emit_pv(ppi, pg, pqc0, pkt, pexps)
                    emit_norm(ppi, pg, pqc0)

                return run

            # ---------------- pipelined emission ----------------
            with (
                tc.tile_pool(name="xtp", bufs=8) as xtpool,
                tc.tile_pool(name="vchp", bufs=3) as vchpool,
                tc.tile_pool(name="ps1", bufs=2, space="PSUM") as ps1,
                tc.tile_pool(name="ps1v", bufs=1, space="PSUM") as ps1v,
                tc.tile_pool(name="ps2", bufs=2, space="PSUM") as ps2,
                tc.tile_pool(name="psrep", bufs=1, space="PSUM") as reppool,
                tc.tile_pool(name="psswp", bufs=1, space="PSUM") as swppool,
            ):
                statps = {}
                for tname in ("q", "k"):
                    statps[(tname, 0)] = ps2.tile(
                        [40, 512], F32, tag="stat", name=f"stat_{tname}0")
                for r in range(4):
                    emit_proj_row(r, xtpool, vchpool, ps1, ps1v, statps)
                emit_statmath("q", 0, statps)
                emit_statmath("k", 0, statps)
                # batch-0 LN interleaved with batch-1 projections
                for tname in ("q", "k"):
                    statps[(tname, 1)] = ps2.tile(
                        [40, 512], F32, tag="stat", name=f"stat_{tname}1")
                for jj in range(4):
                    xts = emit_proj_load(4 + jj, xtpool)
                    cosc, sinc = emit_cs_load(jj)
                    tn_q = emit_apply_pe_head("q", jj, 0, lnbq_sb, reppool)
                    emit_proj_m(4 + jj, 0, xts, vchpool, ps1, ps1v, statps)
                    emit_apply_pe_tail("q", jj, tn_q, cosc, sinc, swppool)
                    emit_proj_m(4 + jj, 1, xts, vchpool, ps1, ps1v, statps)
                    tn_k = emit_apply_pe_head("k", jj, 1, lnbk_sb, reppool)
                    emit_proj_m(4 + jj, 2, xts, vchpool, ps1, ps1v, statps)
                    emit_apply_pe_tail("k", jj, tn_k, cosc, sinc, swppool)
                emit_statmath("q", 1, statps)
                emit_statmath("k", 1, statps)

            # deferred constant loads (keep startup queues clear)
            nc.sync.dma_start(out=wo_sb[:], in_=wo_d.ap()[:, :, :])
            nc.sync.dma_start(out=borep_sb[:], in_=borep_d.ap()[:, :])

            # AllToAll buffers (staged per-pair inside the SDPA stream)
            a2a_in = dpool.tile([NCORE, 128, 512], BF16)
            a2a_out = dpool.tile([NCORE, 128, 512], BF16)

            # batch-1 LN interleaved with batch-0 SDPA
            with (
                tc.tile_pool(name="expp", bufs=4) as exppool,
                tc.tile_pool(name="sp3", bufs=2) as sp3,
                tc.tile_pool(name="sp4", bufs=2) as sp4,
                tc.tile_pool(name="ps_sc", bufs=2, space="PSUM") as ps_sc,
                tc.tile_pool(name="ps_ctx", bufs=4, space="PSUM") as ps_ctx,
            ):
                emit_apply_gp("q", 0, lnwq_sb, lnbq_sb)
                emit_apply_gp("k", 0, lnwk_sb, lnbk_sb)
                emit_apply_gp("q", 1, lnwq_sb, lnbq_sb)
                run_sdpa = make_sdpa(exppool, sp3, sp4, ps_sc, ps_ctx,
                                     a2a_in)
                run_sdpa(
                    [(0, 0), (0, 2), (1, 0), (1, 2)],
                    {
                        0: (lambda: emit_apply_gp("k", 1, lnwk_sb, lnbk_sb),
                            lambda: emit_apply_gp("q", 2, lnwq_sb, lnbq_sb),
                            lambda: emit_apply_gp("k", 2, lnwk_sb, lnbk_sb)),
                        1: (lambda: emit_apply_gp("q", 3, lnwq_sb, lnbq_sb),
                            lambda: emit_apply_gp("k", 3, lnwk_sb, lnbk_sb)),
                    })

            # ================= AllToAll (bf16) =================
            nc.gpsimd.collective_compute(
                "AllToAll", mybir.AluOpType.bypass,
                ins=[a2a_in.opt()], outs=[a2a_out.opt()],
                replica_groups=[list(range(NCORE))],
            )

            # ================= output projection (bf16) ==============
            with (
                tc.tile_pool(name="wop", bufs=3) as wopool,
                tc.tile_pool(name="sp5", bufs=4) as sp5,
                tc.tile_pool(name="ps_out", bufs=4, space="PSUM") as ps_out,
            ):
                ops = [ps_out.tile([128, 1024], F32, tag="outp",
                                   name=f"outp{i}") for i in range(4)]
                for kt in range(KT_DIM):
                    cg = wopool.tile([128, 512], BF16, tag="ctxg",
                                     name=f"cg{kt}")
                    nc.sync.dma_start(out=cg[:], in_=a2a_out[kt])
                    for rt in range(4):
                        for nh in range(2):
                            nc.tensor.matmul(
                                ops[rt][:, nh * 512:(nh + 1) * 512],
                                cg[:, rt * 128:(rt + 1) * 128],
                                wo_sb[:, kt, nh * 512:(nh + 1) * 512],
                                start=(kt == 0), stop=(kt == KT_DIM - 1))
                for rt in range(4):
                    osb = sp5.tile([128, DIM], F32, tag="osb",
                                   name=f"osb{rt}")
                    nc.vector.tensor_tensor(osb[:], ops[rt][:], borep_sb[:],
                                            ADD)
                    nc.sync.dma_start(
                        out=out_d.ap()[rt * 128:(rt + 1) * 128, :],
                        in_=osb[:])

            if DEBUG_OUTPUTS:
                for g in range(B):
                    nc.sync.dma_start(
                        out=dbg_qrot.ap()[:, g * N:(g + 1) * N],
                        in_=qkt[("q", g)][:].bitcast(F32))
                    nc.sync.dma_start(
                        out=dbg_krot.ap()[:, g * N:(g + 1) * N],
                        in_=qkt[("k", g)][:].bitcast(F32))
                nc.gpsimd.dma_start(out=dbg_ctxn.ap()[0:64, :],
                                    in_=ctxn_a[:])
                nc.gpsimd.dma_start(out=dbg_ctxn.ap()[64:128, :],
                                    in_=ctxn_b[:])

    nc.compile()
    return nc


# ---------------------------------------------------------------- host side
def prepare_in_maps(x, rotary_cos, rotary_sin, Wq, bq, Wk, bk, Wv, bv,
                    q_norm_w, q_norm_b, k_norm_w, k_norm_b, Wo, bo):
    import ml_dtypes

    x = np.asarray(x, np.float32)
    xT = _round_fp32r(np.ascontiguousarray(x.reshape(R, DIM).T))

    Wcat = np.concatenate([np.asarray(Wq, np.float32),
                           np.asarray(Wk, np.float32),
                           np.asarray(Wv, np.float32)], axis=1)
    bcat = np.concatenate([np.asarray(bq, np.float32),
                           np.asarray(bk, np.float32),
                           np.asarray(bv, np.float32)])

    def head_cols(h, part):
        s = 192 * h + 64 * part
        return np.arange(s, s + 64)

    cos_flat = np.asarray(rotary_cos, np.float32).reshape(R, HD).T
    sin_flat = np.asarray(rotary_sin, np.float32).reshape(R, HD).T
    sinm = sin_flat.copy()
    sinm[0:32] = -sin_flat[0:32]
    cos_rep = np.ascontiguousarray(np.tile(cos_flat, (2, 1)))
    sinm_rep = np.ascontiguousarray(np.tile(sinm, (2, 1)))

    # stats lhsT, pre-permuted to [128, RC, 2, 40]
    onesblk = np.zeros((128, RC, 2, 40), np.float32)
    for j in range(RC):
        jj = j % 4
        onesblk[0:64, j, 0, 2 * jj] = 1.0
        onesblk[64:128, j, 0, 2 * jj + 1] = 1.0
        onesblk[0:64, j, 1, 32 + 2 * jj] = 1.0
        onesblk[64:128, j, 1, 32 + 2 * jj + 1] = 1.0

    # LN biases [2, 128, 1] and weight-masked broadcast lhsT [2, 2, 128]
    lnb = np.stack([
        np.tile(np.asarray(q_norm_b, np.float32), 2)[:, None],
        np.tile(np.asarray(k_norm_b, np.float32), 2)[:, None],
    ])
    lnw = np.stack([
        np.tile(np.asarray(q_norm_w, np.float32), 2)[:, None],
        np.tile(np.asarray(k_norm_w, np.float32), 2)[:, None],
    ])
    wbc = np.zeros((8, 2, 4, 128), np.float32)
    for i, w in enumerate((q_norm_w, k_norm_w)):
        wt = np.tile(np.asarray(w, np.float32), 2)
        for jj in range(4):
            wbc[2 * jj, i, jj, 0:64] = wt[0:64]
            wbc[2 * jj + 1, i, jj, 64:128] = wt[64:128]
    wbc = _round_fp32r(wbc)

    # RoPE half-swap permutation: out partition p reads src perm_src[p]
    perm = np.zeros((128, 128), np.float32)
    for p in range(128):
        src = p + 32 if (p % 64) < 32 else p - 32
        perm[src, p] = 1.0
    perm = _round_fp32r(perm)

    ident = np.eye(128, dtype=np.float32)
    ones64 = np.ones((128, 4 * NKT), np.float32)
    borep = np.tile(np.asarray(bo, np.float32)[None, :], (128, 1))
    # Wo pre-permuted to [128, KT_DIM, DIM] bf16
    wo_r = np.ascontiguousarray(
        np.asarray(Wo, np.float32).reshape(KT_DIM, 128, DIM)
        .transpose(1, 0, 2)).astype(ml_dtypes.bfloat16)

    in_maps = []
    for c in range(NCORE):
        hA, hB = 2 * c, 2 * c + 1
        cols = np.concatenate([
            head_cols(hA, 0), head_cols(hB, 0),
            head_cols(hA, 1), head_cols(hB, 1),
            head_cols(hA, 2), head_cols(hB, 2),
        ])
        wqkv_c = _round_fp32r(np.ascontiguousarray(
            Wcat[:, cols].reshape(KT_DIM, 128, 384).transpose(1, 0, 2)))
        bqkv_c = np.ascontiguousarray(bcat[cols].reshape(3, 128, 1))
        in_maps.append({
            "xT": xT,
            "wqkv": wqkv_c,
            "bqkv": bqkv_c,
            "onesblk": onesblk,
            "lnb": lnb,
            "lnw": lnw,
            "wbc": wbc,
            "perm": perm,
            "cosr": cos_rep,
            "sinm": sinm_rep,
            "ident": ident,
            "ones64": ones64,
            "wo": wo_r,
            "borep": borep,
        })
    return in_maps


def assemble_output(results):
    out = np.empty((R, DIM), np.float32)
    for c in range(NCORE):
        out[c * 512:(c + 1) * 512] = results[c]["out"]
    return out.reshape(B, N, DIM)


_NC_CACHE = []


def kernel(**inputs) -> np.ndarray:
    if not _NC_CACHE:
        _NC_CACHE.append(build())
    nc = _NC_CACHE[0]
    in_maps = prepare_in_maps(**inputs)
    res = run_bass_kernel_spmd(nc, in_maps, core_ids=list(range(NCORE)))
    return assemble_output(res.results)
